# revision 12
# baseline (speedup 1.0000x reference)
"""CrossSpatialAttention Trainium2 kernel.

Reference computation (per batch b, N = D*H*W = 8192 tokens, C=256, MID=64):
  f = relu(bn_f(Wf x)), g = relu(bn_g(Wg x)), h = Wh x          [MID, N]
  attn = softmax_m(f^T g / sqrt(MID))                            [N, N]
  z = attn @ h^T -> [MID, N];  out = Wv z + bv + x               [C, N]

Sharding: 8 cores = (batch b in {0,1}) x (query chunk of 2048 tokens).
Each core gets the full x for its batch (keys/values need all tokens) and
computes attention output for its 2048 queries.

Per-core kernel (Tile framework):
  - BN is folded into Wf/Wg + per-channel bias on host.
  - h/v biases commute through softmax (rows sum to 1), folded into a single
    per-output-channel bias bo = Wv @ bh + bv applied at the end.
  - S^T layout: for each key block m (128 keys), S^T[m, q] = (g_m)^T f via PE,
    exp on ScalarE (logits are >= 0 and <= ~14.1 for these inputs, so no
    max-subtraction is needed; exp and its 8192-term sum stay in fp32 range),
    then O^T[c, q] += [h^T | 1]_m @ P_m accumulated in PSUM -- the appended
    ones column produces the softmax denominator in row MID.
"""

import numpy as np

B, C, N = 2, 256, 8192
MID = 64
NCORES = 8
QC = N // 4            # queries per core (2048)
QT = 1024              # query tile (psum-sized)
EPS = 1e-5
SCALE = float(MID) ** -0.5
MB = 128               # key block
NMB = N // MB          # 64 key blocks

USE_F32R = False       # fp32r matmuls: 1 cyc/row (vs 4 for fp32) when N>=256

_cache = {}


def _build():
    import concourse.bacc as bacc
    import concourse.tile as tile
    from concourse import mybir

    f32 = mybir.dt.float32
    f32r = mybir.dt.float32r
    AF = mybir.ActivationFunctionType

    def mm(ap):
        return ap.bitcast(f32r) if USE_F32R else ap

    nc = bacc.Bacc(trn_type="TRN2", target_bir_lowering=False, debug=False)

    xb = nc.dram_tensor("xb", [C, N], f32, kind="ExternalInput").ap()
    xq = nc.dram_tensor("xq", [C, QC], f32, kind="ExternalInput").ap()
    wfT = nc.dram_tensor("wfT", [2, 128, MID], f32, kind="ExternalInput").ap()
    wgT = nc.dram_tensor("wgT", [2, 128, MID], f32, kind="ExternalInput").ap()
    whT = nc.dram_tensor("whT", [2, 128, MID], f32, kind="ExternalInput").ap()
    wvT = nc.dram_tensor("wvT", [MID, C], f32, kind="ExternalInput").ap()
    bf = nc.dram_tensor("bf", [MID, 1], f32, kind="ExternalInput").ap()
    bg = nc.dram_tensor("bg", [MID, 1], f32, kind="ExternalInput").ap()
    bo = nc.dram_tensor("bo", [C, 1], f32, kind="ExternalInput").ap()
    out = nc.dram_tensor("out", [C, QC], f32, kind="ExternalOutput").ap()

    with tile.TileContext(nc) as tc:
        with (
            tc.tile_pool(name="consts", bufs=1) as consts,
            tc.tile_pool(name="xpool", bufs=1) as xpool,
            tc.tile_pool(name="proj", bufs=1) as proj,
            tc.tile_pool(name="ppool", bufs=3) as ppool,
            tc.tile_pool(name="zpool", bufs=2) as zpool,
            tc.tile_pool(name="opool", bufs=3) as opool,
            tc.tile_pool(name="ps_st", bufs=2, space="PSUM") as ps_st,
            tc.tile_pool(name="ps_acc", bufs=1, space="PSUM") as ps_acc,
            tc.tile_pool(name="ps_gen", bufs=2, space="PSUM") as ps_gen,
        ):
            # ---- constants ----
            wf_t = consts.tile([128, 2, MID], f32)
            wg_t = consts.tile([128, 2, MID], f32)
            wh_t = consts.tile([128, 2, MID], f32)
            wv_t = consts.tile([MID, C], f32)
            bf_t = consts.tile([MID, 1], f32)
            bg_t = consts.tile([MID, 1], f32)
            bo_t = consts.tile([128, C // 128, 1], f32)
            nc.gpsimd.dma_start(out=wf_t, in_=wfT.rearrange("k p m -> p k m"))
            nc.gpsimd.dma_start(out=wg_t, in_=wgT.rearrange("k p m -> p k m"))
            nc.gpsimd.dma_start(out=wh_t, in_=whT.rearrange("k p m -> p k m"))
            nc.gpsimd.dma_start(out=wv_t, in_=wvT)
            nc.gpsimd.dma_start(out=bf_t, in_=bf)
            nc.gpsimd.dma_start(out=bg_t, in_=bg)
            nc.gpsimd.dma_start(out=bo_t, in_=bo.rearrange("(o p) x -> p o x", p=128))

            # ---- x tiles: full batch [128, 2, N], queries [128, 2, QC] ----
            x_t = xpool.tile([128, 2, N], f32)
            xq_t = xpool.tile([128, 2, QC], f32)
            xb_r = xb.rearrange("(k p) n -> k p n", p=128)
            xq_r = xq.rearrange("(k p) n -> k p n", p=128)
            for k in range(2):
                for half in range(2):
                    sl = slice(half * (N // 2), (half + 1) * (N // 2))
                    nc.sync.dma_start(out=x_t[:, k, sl], in_=xb_r[k, :, sl])
                nc.sync.dma_start(out=xq_t[:, k, :], in_=xq_r[k, :, :])

            # ---- priming matmuls: absorb each DMA semaphore into PE's
            # observed clock so no later matmul needs >1 sync wait (walrus
            # caps fused-matmul waits at 1) ----
            prime_srcs = [wf_t[:, 0, :], wg_t[:, 0, :], wh_t[:, 0, :],
                          wv_t[:, 0:MID]]
            for k in range(2):
                for half in range(2):
                    o = half * (N // 2)
                    prime_srcs.append(x_t[:, k, o:o + MID])
                prime_srcs.append(xq_t[:, k, 0:MID])
            dp = ps_st.tile([128, QT], f32, tag="st")
            for i, src in enumerate(prime_srcs):
                nc.tensor.matmul(dp[0:src.shape[-1], 0:MID], src, src[:, 0:MID],
                                 start=(i == 0), stop=(i == len(prime_srcs) - 1),
                                 skip_group_check=True)
            trash = ppool.tile([128, QT], f32, tag="p")
            nc.scalar.activation(trash, dp, AF.Copy, bias=0.0, scale=1.0)

            # ---- projections ----
            g_t = proj.tile([MID, N], f32)
            f_t = proj.tile([MID, QC], f32)
            hTo = proj.tile([128, NMB, MID + 1], f32)  # [m, block, c|1]

            # ones column via ScalarE so every hTo producer is on ACT
            nc.scalar.activation(hTo[:, :, MID], wg_t[:, 0, :],
                                 AF.Copy, bias=1.0, scale=0.0)

            # g = relu(Wg' x + bg'), full N
            for n in range(N // 512):
                sl = slice(n * 512, (n + 1) * 512)
                pg = ps_gen.tile([MID, 512], f32, tag="pg")
                for k in range(2):
                    nc.tensor.matmul(pg, mm(wg_t[:, k, :]), mm(x_t[:, k, sl]),
                                     start=(k == 0), stop=(k == 1))
                nc.scalar.activation(g_t[:, sl], pg, AF.Relu, bias=bg_t, scale=1.0)
            # f = relu(Wf' xq + bf'), QC queries
            for n in range(QC // 512):
                sl = slice(n * 512, (n + 1) * 512)
                pf = ps_gen.tile([MID, 512], f32, tag="pg")
                for k in range(2):
                    nc.tensor.matmul(pf, mm(wf_t[:, k, :]), mm(xq_t[:, k, sl]),
                                     start=(k == 0), stop=(k == 1))
                nc.scalar.activation(f_t[:, sl], pf, AF.Relu, bias=bf_t, scale=1.0)
            # hT[m, c] = x^T Wh^T, one accumulation region per psum tile
            for mb in range(NMB):
                ph = ps_gen.tile([128, MID], f32, tag="pg")
                msl = slice(mb * MB, (mb + 1) * MB)
                for k in range(2):
                    nc.tensor.matmul(ph, mm(x_t[:, k, msl]), mm(wh_t[:, k, :]),
                                     start=(k == 0), stop=(k == 1))
                nc.scalar.activation(hTo[:, mb, 0:MID], ph, AF.Copy,
                                     bias=0.0, scale=1.0)

            # ---- attention ----
            for qi in range(QC // QT):
                qsl = slice(qi * QT, (qi + 1) * QT)
                o_ps = ps_acc.tile([MID + 1, QT], f32, tag="acc")
                for mb in range(NMB):
                    msl = slice(mb * MB, (mb + 1) * MB)
                    st = ps_st.tile([128, QT], f32, tag="st")
                    for h in range(QT // 512):
                        fs = slice(qi * QT + h * 512, qi * QT + (h + 1) * 512)
                        nc.tensor.matmul(st[:, h * 512:(h + 1) * 512],
                                         mm(g_t[:, msl]), mm(f_t[:, fs]),
                                         start=True, stop=True)
                    p_t = ppool.tile([128, QT], f32, tag="p")
                    nc.scalar.activation(p_t, st, AF.Exp, scale=SCALE)
                    for h in range(QT // 512):
                        hs = slice(h * 512, (h + 1) * 512)
                        nc.tensor.matmul(o_ps[:, hs], mm(hTo[:, mb, :]),
                                         mm(p_t[:, hs]),
                                         start=(mb == 0), stop=(mb == NMB - 1))

                # normalize: z = O / denom
                rd = zpool.tile([1, QT], f32, tag="rd")
                nc.vector.reciprocal(rd, o_ps[MID:MID + 1, :])
                rb = zpool.tile([MID, QT], f32, tag="rb")
                nc.gpsimd.partition_broadcast(rb, rd)
                z_t = zpool.tile([MID, QT], f32, tag="z")
                nc.vector.tensor_mul(z_t, o_ps[0:MID, :], rb)

                # out = Wv z + bo + xq
                for oh in range(C // 128):
                    osl = slice(oh * 128, (oh + 1) * 128)
                    for h in range(QT // 512):
                        hs = slice(h * 512, (h + 1) * 512)
                        po = ps_gen.tile([128, 512], f32, tag="pg")
                        nc.tensor.matmul(po, mm(wv_t[:, osl]), mm(z_t[:, hs]),
                                         start=True, stop=True)
                        o_sb = opool.tile([128, 512], f32, tag="ob")
                        nc.vector.tensor_scalar_add(o_sb, po, bo_t[:, oh, :])
                        qs = slice(qi * QT + h * 512, qi * QT + (h + 1) * 512)
                        nc.vector.tensor_add(o_sb, o_sb, xq_t[:, oh, qs])
                        nc.sync.dma_start(out=out.rearrange(
                            "(o p) n -> o p n", p=128)[oh, :, qs], in_=o_sb)

    nc.compile()
    return nc


def _prep_inputs(inputs):
    f32 = np.float32
    x = np.asarray(inputs["x"], f32).reshape(B, C, N)

    def fold(W, b, gam, bet, m, v):
        inv = np.asarray(gam, f32) / np.sqrt(np.asarray(v, f32) + EPS)
        We = np.asarray(W, f32) * inv[:, None]
        be = np.asarray(b, f32) * inv + np.asarray(bet, f32) - np.asarray(m, f32) * inv
        return We, be

    Wf, bfe = fold(inputs["Wf"], inputs["bf"], inputs["gamf"], inputs["betf"],
                   inputs["mf"], inputs["vf"])
    Wg, bge = fold(inputs["Wg"], inputs["bg"], inputs["gamg"], inputs["betg"],
                   inputs["mg"], inputs["vg"])
    Wh = np.asarray(inputs["Wh"], f32)
    Wv = np.asarray(inputs["Wv"], f32)
    bo = Wv @ np.asarray(inputs["bh"], f32) + np.asarray(inputs["bv"], f32)

    wfT = np.ascontiguousarray(Wf.T.reshape(2, 128, MID))
    wgT = np.ascontiguousarray(Wg.T.reshape(2, 128, MID))
    whT = np.ascontiguousarray(Wh.T.reshape(2, 128, MID))
    wvT = np.ascontiguousarray(Wv.T)

    in_maps = []
    for core in range(NCORES):
        b, qc = divmod(core, 4)
        in_maps.append({
            "xb": np.ascontiguousarray(x[b]),
            "xq": np.ascontiguousarray(x[b][:, qc * QC:(qc + 1) * QC]),
            "wfT": wfT, "wgT": wgT, "whT": whT, "wvT": wvT,
            "bf": bfe.reshape(MID, 1).copy(),
            "bg": bge.reshape(MID, 1).copy(),
            "bo": bo.reshape(C, 1).copy(),
        })
    return in_maps


def _run(inputs, trace=False, **kw):
    from concourse.bass_utils import run_bass_kernel_spmd

    if "nc" not in _cache:
        _cache["nc"] = _build()
    in_maps = _prep_inputs(inputs)
    br = run_bass_kernel_spmd(_cache["nc"], in_maps, list(range(NCORES)),
                              trace=trace, **kw)
    out = np.empty((B, C, N), np.float32)
    for core in range(NCORES):
        b, qc = divmod(core, 4)
        out[b][:, qc * QC:(qc + 1) * QC] = br.results[core]["out"]
    return out.reshape(B, C, 8, 32, 32), br


def kernel(**inputs):
    out, _ = _run(inputs)
    return out
